# revision 9
# baseline (speedup 1.0000x reference)
"""Trainium2 Bass kernel for per-head L2-distance attention + grouped output
projection + BatchNorm (dense_transformer, B=2, dim=256, N=3072, H=8, D=32).

Sharding: one head per NeuronCore (8 heads = 8 cores), both batches on each
core.  Channels split by head, so the BatchNorm per-channel (b, n) reduction
is fully core-local -> zero collectives.

v2 design (vs the fp32r baseline):
  - All N^2 matmuls in bf16 (1 cyc/row vs fp32r's 2 on HW).
  - Weight folding kills the q/k/v materialization almost entirely:
      W    = -2 wq^T wk   (32x32): qk-part of the distance contracts x
             directly against g = W^T x, so only g needs a PSUM->SBUF copy.
      W2   = (wo wv)^T    (32x32): v is never materialized; the PV matmul
             contracts attention against x^T (host-transposed layout), and
             the output projection collapses to one small epilogue matmul.
  - Augmented contraction (K=96): qa = [x; q^2; ones], ka = [g; ones; k^2]
    so ST = sum_K qa*ka = -2qk + q2 + k2 = ||q_i - k_j||^2, ST[j, i].
  - Exp chain is the baseline's exact t-domain pipeline (precision-critical:
    BN divides by a tiny per-channel variance, amplifying any systematic
    attention-weight error ~50x):
      ACT: t = Sqrt(act_scale*ST + act_bias)  (PSUM -> SBUF fp32)
      DVE: p = exppoly(t) ~= exp(-sqrt(d2)/sqrt(32))  (custom op, bf16 out)
  - PV: [x^T | 1] @ p accumulated over j-tiles -> outX[c,i] + rowsum.
  - Epilogue per 1024-i-block: recip rowsum (DVE, DRAM-bounce broadcast),
    normalize outX (DVE), W2 matmul, bn_stats from PSUM.
  - BatchNorm tail re-runs the W2 matmul from the persistent normalized
    outX instead of storing y (saves a PSUM->SBUF pass per block).
"""

import numpy as np
from contextlib import ExitStack

import concourse.bass as bass
import concourse.tile as tile
from concourse import bacc, mybir
from concourse.bass_utils import run_bass_kernel_spmd

F32 = mybir.dt.float32
F32R = mybir.dt.float32r
BF16 = mybir.dt.bfloat16
AFT = mybir.ActivationFunctionType

B, DIM, N, H, D = 2, 256, 3072, 8, 32
C = DIM // H          # 32 input channels per head
NT = N // 128         # 24 j-tiles
NC = N // 512         # 6 512-chunks
NB = N // 1024        # 3 i-blocks per batch
EPS_BN = 1e-5

# --- exp polynomial calibration (baseline-proven, rel err ~1.5e-6) ---
# w = sqrt(d2)/(8*sqrt(32)); minimax cubic for exp(-t/gam) on t in [0, W*gam],
# rescaled so the cubic coefficient is -1:  p(t) = B0 + t*(B1 + t*(B2 - t)),
# out = ((p^2)^2)^2 = exp(-sqrt(d2)/sqrt(32)).
B0 = 0.999999894052468
B1 = -1.858707805584652
B2 = 1.7242982194980068
ACT_SCALE = 0.00014132731  # (gam*scale/8)^2
ACT_BIAS = 1.413273e-09    # ACT_SCALE * 1e-5 protective epsilon inside sqrt

_EXP_OP = None


def _register_exp_op():
    """Register the exp(-.) polynomial as a custom DVE op (in-process)."""
    global _EXP_OP
    if _EXP_OP is not None:
        return _EXP_OP
    import concourse.dve_ops as dve_ops
    from concourse.dve_spec import Spec, Src0, C0, C1, C2, sq, lower, _has_src1
    from concourse.dve_uop import DveOpSpec

    name = "EXP_NEG_POLY3SQ3_ANT"
    for o in dve_ops.OPS:
        if o.name == name:
            _EXP_OP = o
            return o

    t = Src0
    body = sq(sq(sq(C0 + t * (C1 + t * (C2 - t)))))

    def ref(in0, in1, c0, c1, c2):
        tt = in0.astype(np.float32)
        p = (c0 + tt * (c1 + tt * (c2 - tt))).astype(np.float32)
        for _ in range(3):
            p = (p * p).astype(np.float32)
        return p

    spec = Spec(body=body, reference=ref)
    row = dve_ops._CUSTOM_DVE_ROW_BASE + len(dve_ops.OPS)
    shas = {}
    for ver in ("v3", "v4"):
        try:
            uops = lower(spec, ver=ver)
            s = DveOpSpec(name=name, opcode=row, uops=uops, rd1_en=_has_src1(spec))
            shas[ver] = s.sha(ver)
        except Exception:
            pass
    op = dve_ops.DveOp(name, spec, subdim=False, uops_sha=shas)
    dve_ops.OPS.append(op)
    dve_ops._SUB_OPCODE_FOR_NAME[name] = row
    dve_ops.CUSTOM_DVE_SPECS[name] = spec
    _EXP_OP = op
    return op


def _bcast_rows(ap: bass.AP, nrows: int) -> bass.AP:
    """[1, n] AP -> partition-stride-0 [nrows, n] AP (for DMA replicate)."""
    return bass.AP(tensor=ap.tensor, offset=ap.offset, ap=[[0, nrows], ap.ap[-1]])


def build_program():
    exp_op = _register_exp_op()
    nc = bacc.Bacc("TRN2", target_bir_lowering=False, debug=False)

    xh_d = nc.dram_tensor("xh", [B, C, N], BF16, kind="ExternalInput").ap()
    xt_d = nc.dram_tensor("xt", [B, 128, NT, 33], BF16, kind="ExternalInput").ap()
    wall_d = nc.dram_tensor("wall", [C, 96], BF16, kind="ExternalInput").ap()
    w2_d = nc.dram_tensor("w2t", [C, C], F32R, kind="ExternalInput").ap()
    gm_d = nc.dram_tensor("gm", [C, 1], F32, kind="ExternalInput").ap()
    bt_d = nc.dram_tensor("bt", [C, 1], F32, kind="ExternalInput").ap()
    y_d = nc.dram_tensor("y", [B, C, N], F32, kind="ExternalOutput").ap()

    with tile.TileContext(nc) as tc:
        with tc.tile_pool(name="const", bufs=1) as const, \
             tc.tile_pool(name="persist", bufs=1) as persist, \
             tc.tile_pool(name="drp", bufs=2, space="DRAM") as drp:
            wall_s = const.tile([C, 96], BF16)
            w2_s = const.tile([C, C], F32R)
            gm_s = const.tile([C, 1], F32)
            bt_s = const.tile([C, 1], F32)
            actb = const.tile([128, 1], F32)
            epsb = const.tile([C, 1], F32)
            for dst, src in ((wall_s, wall_d), (w2_s, w2_d),
                             (gm_s, gm_d), (bt_s, bt_d)):
                nc.sync.dma_start(out=dst, in_=src)
            nc.vector.memset(actb, ACT_BIAS)
            nc.vector.memset(epsb, EPS_BN)

            outn_sb = [persist.tile([C, N], F32R, tag=f"on{b}", name=f"outn{b}")
                       for b in range(B)]
            stats = persist.tile([C, B * NC, 6], F32)

            # Emit both batches' prologues first; the Tile scheduler overlaps
            # b=1 prologue with b=0 main loop.
            pro = {}
            es = ExitStack()
            xbp = es.enter_context(tc.tile_pool(name="xb", bufs=1))
            pprojp = es.enter_context(tc.tile_pool(name="pproj", bufs=2,
                                                   space="PSUM"))
            for b in range(B):
                qa = xbp.tile([96, N], BF16, tag=f"qa{b}", name=f"qa{b}")
                ka = xbp.tile([96, N], BF16, tag=f"ka{b}", name=f"ka{b}")
                xaT = xbp.tile([128, NT, 33], BF16, tag=f"xaT{b}",
                               name=f"xaT{b}")
                # x straight into qa's qk-rows; x^T (with ones col) from host
                for xc in range(NC):
                    xsl = bass.ts(xc, 512)
                    nc.gpsimd.dma_start(out=qa[0:32, xsl], in_=xh_d[b][:, xsl])
                nc.sync.dma_start(out=xaT, in_=xt_d[b])
                nc.gpsimd.memset(qa[64:96, :], 1.0)
                nc.gpsimd.memset(ka[32:64, :], 1.0)
                # combined projection: rows 0-31 g = W^T x, 32-63 q, 64-95 k
                for icx in range(NC):
                    sl = bass.ts(icx, 512)
                    ps_all = pprojp.tile([96, 512], F32, tag="pp",
                                         name="ps_all")
                    nc.tensor.matmul(ps_all, lhsT=wall_s, rhs=qa[0:32, sl],
                                     start=True, stop=True)
                    nc.scalar.copy(ka[0:32, sl], ps_all[0:32, :])
                    nc.scalar.activation(qa[32:64, sl], ps_all[32:64, :],
                                         AFT.Square, bias=0.0, scale=1.0)
                    nc.scalar.activation(ka[64:96, sl], ps_all[64:96, :],
                                         AFT.Square, bias=0.0, scale=1.0)
                pro[b] = (qa, ka, xaT)

            # main attention loops: i-blocks of 1024 (2 psum banks)
            with tc.tile_pool(name="mt", bufs=3) as mt, \
                 tc.tile_pool(name="ps_st", bufs=2, space="PSUM") as ps_st, \
                 tc.tile_pool(name="ps_acc", bufs=1, space="PSUM") as ps_acc, \
                 tc.tile_pool(name="ep", bufs=2) as ep:
                for b in range(B):
                    qa, ka, xaT = pro[b]
                    for icb in range(NB):
                        outX_ps = ps_acc.tile([33, 1024], F32, tag="outX",
                                              name="outX_ps")

                        # software-pipelined: PV matmuls for pair k are
                        # emitted after pair k+1's ST matmuls so the PE queue
                        # never stalls waiting on the ACT->DVE chain.
                        pending = []

                        def flush_pv(outX_ps=outX_ps):
                            pj, p_sb = pending.pop(0)
                            for half in range(2):
                                jt = pj * 2 + half
                                for hh in range(2):
                                    nc.tensor.matmul(
                                        outX_ps[:, bass.ts(hh, 512)],
                                        lhsT=xaT[:, jt, :],
                                        rhs=p_sb[:, bass.ds(half * 1024 +
                                                            hh * 512, 512)],
                                        start=(jt == 0),
                                        stop=(jt == NT - 1))

                        for pj in range(NT // 2):
                            t_sb = mt.tile([128, 2048], F32, tag="t",
                                           name="t_sb", bufs=2)
                            for half in range(2):
                                jt = pj * 2 + half
                                st_ps = ps_st.tile([128, 1024], F32, tag="st",
                                                   name="st_ps")
                                for hh in range(2):
                                    nsl = bass.ds(icb * 1024 + hh * 512, 512)
                                    nc.tensor.matmul(
                                        st_ps[:, bass.ts(hh, 512)],
                                        lhsT=ka[:, bass.ts(jt, 128)],
                                        rhs=qa[:, nsl],
                                        start=True, stop=True)
                                nc.scalar.activation(
                                    t_sb[:, bass.ts(half, 1024)], st_ps,
                                    AFT.Sqrt, bias=actb, scale=ACT_SCALE)
                            p_sb = mt.tile([128, 2048], BF16, tag="p",
                                           name="p_sb", bufs=2)
                            nc.vector._custom_dve(exp_op, out=p_sb, in0=t_sb,
                                                  s0=B0, s1=B1, imm2=B2)
                            pending.append((pj, p_sb))
                            if len(pending) > 1:
                                flush_pv()
                        while pending:
                            flush_pv()
                        # epilogue for this i-block: one ACT pass brings outX
                        # (incl rowsum row) to SBUF; recip runs on the [128,8]
                        # layout via DMA reshape; normalize on GPSIMD.
                        outc = ep.tile([33, 1024], F32, tag="outc",
                                       name="outc")
                        nc.scalar.copy(outc, outX_ps)
                        r128 = ep.tile([128, 8], F32, tag="r128", name="r128")
                        nc.sync.dma_start(out=r128, in_=outc[32:33, :])
                        recip = ep.tile([128, 8], F32, tag="recip",
                                        name="recip")
                        nc.vector.reciprocal(recip, r128)
                        rdr = drp.tile([1, 1024], F32, tag="rdr", name="rdr")
                        nc.sync.dma_start(out=rdr, in_=recip)
                        rbc = ep.tile([C, 1024], F32, tag="rbc", name="rbc")
                        nc.sync.dma_start(out=rbc, in_=_bcast_rows(rdr, C))
                        osl = bass.ds(icb * 1024, 1024)
                        nc.gpsimd.tensor_mul(outn_sb[b][:, osl],
                                             outc[0:32, :], rbc)
                        for hh in range(2):
                            hsl = bass.ds(icb * 1024 + hh * 512, 512)
                            y_ps = pprojp.tile([C, 512], F32, tag="pp",
                                               name="y_ps")
                            nc.tensor.matmul(y_ps, lhsT=w2_s,
                                             rhs=outn_sb[b][:, hsl],
                                             start=True, stop=True)
                            nc.vector.bn_stats(
                                stats[:, (b * NB + icb) * 2 + hh, :], y_ps)
            es.close()

            # BatchNorm tail: aggregate stats, re-project, affine, store
            with tc.tile_pool(name="tail", bufs=1) as tail, \
                 tc.tile_pool(name="ps_tl", bufs=2, space="PSUM") as ps_tl:
                mv = tail.tile([C, 2], F32)
                nc.vector.bn_aggr(mv, stats)
                std = tail.tile([C, 1], F32)
                nc.scalar.activation(std, mv[:, 1:2], AFT.Sqrt,
                                     bias=epsb, scale=1.0)
                rstd = tail.tile([C, 1], F32)
                nc.vector.reciprocal(rstd, std)
                sc = tail.tile([C, 1], F32)
                nc.vector.tensor_mul(sc, gm_s, rstd)
                msc = tail.tile([C, 1], F32)
                nc.vector.tensor_mul(msc, mv[:, 0:1], sc)
                nb = tail.tile([C, 1], F32)
                nc.vector.tensor_sub(nb, bt_s, msc)
                for b in range(B):
                    for icx in range(NC):
                        sl = bass.ts(icx, 512)
                        yt_ps = ps_tl.tile([C, 512], F32, tag="yt",
                                           name="yt_ps")
                        nc.tensor.matmul(yt_ps, lhsT=w2_s,
                                         rhs=outn_sb[b][:, sl],
                                         start=True, stop=True)
                        yo = tail.tile([C, 512], F32, tag="yo", name="yo",
                                       bufs=4)
                        if icx % 2 == 0:
                            nc.scalar.activation(yo, yt_ps, AFT.Identity,
                                                 bias=nb, scale=sc)
                        else:
                            nc.vector.tensor_scalar(
                                out=yo, in0=yt_ps, scalar1=sc, scalar2=nb,
                                op0=mybir.AluOpType.mult,
                                op1=mybir.AluOpType.add)
                        nc.sync.dma_start(out=y_d[b][:, sl], in_=yo)

    nc.compile()
    return nc


_NC_CACHE = None


def _get_nc():
    global _NC_CACHE
    if _NC_CACHE is None:
        _NC_CACHE = build_program()
    return _NC_CACHE


def make_in_maps(x, wq, wk, wv, wo, gamma, beta):
    import ml_dtypes
    f = np.float32
    bf = ml_dtypes.bfloat16
    in_maps = []
    for h in range(H):
        cs = slice(h * C, (h + 1) * C)
        xh = np.ascontiguousarray(x[:, cs, :]).astype(bf)         # [B, C, N]
        # x^T tiles with ones column: [B, 128, NT, 33]
        xt = np.ones((B, 128, NT, 33), f)
        xtr = np.ascontiguousarray(x[:, cs, :].transpose(0, 2, 1))  # [B,N,C]
        xt[:, :, :, :32] = xtr.reshape(B, NT, 128, C).transpose(0, 2, 1, 3)
        wall = np.empty((C, 96), f)
        wall[:, 0:32] = -2.0 * (wk[h].T @ wq[h])    # g-rows weight (G.T)
        wall[:, 32:64] = wq[h].T
        wall[:, 64:96] = wk[h].T
        in_maps.append({
            "xh": xh,
            "xt": xt.astype(bf),
            "wall": wall.astype(bf),
            "w2t": np.ascontiguousarray((wo[h] @ wv[h]).T.astype(f)),
            "gm": np.ascontiguousarray(gamma[cs].reshape(C, 1).astype(f)),
            "bt": np.ascontiguousarray(beta[cs].reshape(C, 1).astype(f)),
        })
    return in_maps


def kernel(x, wq, wk, wv, wo, bo, gamma, beta):
    x, wq, wk, wv, wo, gamma, beta = (np.asarray(a) for a in
                                      (x, wq, wk, wv, wo, gamma, beta))
    nc = _get_nc()
    in_maps = make_in_maps(x, wq, wk, wv, wo, gamma, beta)
    res = run_bass_kernel_spmd(nc, in_maps, list(range(H)))
    y = np.empty((B, DIM, N), np.float32)
    for h in range(H):
        y[:, h * C:(h + 1) * C, :] = res.results[h]["y"]
    return y


# revision 19
# speedup vs baseline: 1.0198x; 1.0198x over previous
"""Trainium2 Bass kernel for per-head L2-distance attention + grouped output
projection + BatchNorm (dense_transformer, B=2, dim=256, N=3072, H=8, D=32).

Sharding: one head per NeuronCore (8 heads = 8 cores), both batches on each
core.  Channels split by head, so the BatchNorm per-channel (b, n) reduction
is fully core-local -> zero collectives.

v2 design (vs the fp32r baseline):
  - All N^2 matmuls in bf16 (1 cyc/row vs fp32r's 2 on HW).
  - Weight folding kills the q/k/v materialization almost entirely:
      W    = -2 wq^T wk   (32x32): qk-part of the distance contracts x
             directly against g = W^T x, so only g needs a PSUM->SBUF copy.
      W2   = (wo wv)^T    (32x32): v is never materialized; the PV matmul
             contracts attention against x^T (host-transposed layout), and
             the output projection collapses to one small epilogue matmul.
  - Augmented contraction (K=96): qa = [x; q^2; ones], ka = [g; ones; k^2]
    so ST = sum_K qa*ka = -2qk + q2 + k2 = ||q_i - k_j||^2, ST[j, i].
  - Exp chain is the baseline's exact t-domain pipeline (precision-critical:
    BN divides by a tiny per-channel variance, amplifying any systematic
    attention-weight error ~50x):
      ACT: t = Sqrt(act_scale*ST + act_bias)  (PSUM -> SBUF fp32)
      DVE: p = exppoly(t) ~= exp(-sqrt(d2)/sqrt(32))  (custom op, bf16 out)
  - PV: [x^T | 1] @ p accumulated over j-tiles -> outX[c,i] + rowsum.
  - Epilogue per 1024-i-block: recip rowsum (DVE, DRAM-bounce broadcast),
    normalize outX (DVE), W2 matmul, bn_stats from PSUM.
  - BatchNorm tail re-runs the W2 matmul from the persistent normalized
    outX instead of storing y (saves a PSUM->SBUF pass per block).
"""

import numpy as np
from contextlib import ExitStack

import concourse.bass as bass
import concourse.tile as tile
from concourse import bacc, mybir
from concourse.bass_utils import run_bass_kernel_spmd

F32 = mybir.dt.float32
F32R = mybir.dt.float32r
BF16 = mybir.dt.bfloat16
AFT = mybir.ActivationFunctionType

B, DIM, N, H, D = 2, 256, 3072, 8, 32
C = DIM // H          # 32 input channels per head
NT = N // 128         # 24 j-tiles
NC = N // 512         # 6 512-chunks
NB = N // 1024        # 3 i-blocks per batch
EPS_BN = 1e-5

# --- exp polynomial calibration (baseline-proven, rel err ~1.5e-6) ---
# w = sqrt(d2)/(8*sqrt(32)); minimax cubic for exp(-t/gam) on t in [0, W*gam],
# rescaled so the cubic coefficient is -1:  p(t) = B0 + t*(B1 + t*(B2 - t)),
# out = ((p^2)^2)^2 = exp(-sqrt(d2)/sqrt(32)).
B0 = 0.999999894052468
B1 = -1.858707805584652
B2 = 1.7242982194980068
ACT_SCALE = 0.00014132731  # (gam*scale/8)^2
ACT_BIAS = 1.413273e-09    # ACT_SCALE * 1e-5 protective epsilon inside sqrt

_EXP_OP = None
_SQ_OP = None


def _register_sq_op():
    """out = in0^2 as a custom DVE op (single PSUM read, unlike tensor_mul)."""
    global _SQ_OP
    if _SQ_OP is not None:
        return _SQ_OP
    import concourse.dve_ops as dve_ops
    from concourse.dve_spec import Spec, Src0, sq, lower, _has_src1
    from concourse.dve_uop import DveOpSpec

    name = "SQUARE_SRC0_ANT"
    for o in dve_ops.OPS:
        if o.name == name:
            _SQ_OP = o
            return o

    spec = Spec(body=sq(Src0),
                reference=lambda in0, in1, c0, c1, c2:
                    (in0.astype(np.float32) ** 2).astype(np.float32))
    row = dve_ops._CUSTOM_DVE_ROW_BASE + len(dve_ops.OPS)
    shas = {}
    for ver in ("v3", "v4"):
        try:
            uops = lower(spec, ver=ver)
            s = DveOpSpec(name=name, opcode=row, uops=uops,
                          rd1_en=_has_src1(spec))
            shas[ver] = s.sha(ver)
        except Exception:
            pass
    op = dve_ops.DveOp(name, spec, subdim=False, uops_sha=shas)
    dve_ops.OPS.append(op)
    dve_ops._SUB_OPCODE_FOR_NAME[name] = row
    dve_ops.CUSTOM_DVE_SPECS[name] = spec
    _SQ_OP = op
    return op


def _register_exp_op():
    """Register the exp(-.) polynomial as a custom DVE op (in-process)."""
    global _EXP_OP
    if _EXP_OP is not None:
        return _EXP_OP
    import concourse.dve_ops as dve_ops
    from concourse.dve_spec import Spec, Src0, C0, C1, C2, sq, lower, _has_src1
    from concourse.dve_uop import DveOpSpec

    name = "EXP_NEG_POLY3SQ3_ANT"
    for o in dve_ops.OPS:
        if o.name == name:
            _EXP_OP = o
            return o

    t = Src0
    body = sq(sq(sq(C0 + t * (C1 + t * (C2 - t)))))

    def ref(in0, in1, c0, c1, c2):
        tt = in0.astype(np.float32)
        p = (c0 + tt * (c1 + tt * (c2 - tt))).astype(np.float32)
        for _ in range(3):
            p = (p * p).astype(np.float32)
        return p

    spec = Spec(body=body, reference=ref)
    row = dve_ops._CUSTOM_DVE_ROW_BASE + len(dve_ops.OPS)
    shas = {}
    for ver in ("v3", "v4"):
        try:
            uops = lower(spec, ver=ver)
            s = DveOpSpec(name=name, opcode=row, uops=uops, rd1_en=_has_src1(spec))
            shas[ver] = s.sha(ver)
        except Exception:
            pass
    op = dve_ops.DveOp(name, spec, subdim=False, uops_sha=shas)
    dve_ops.OPS.append(op)
    dve_ops._SUB_OPCODE_FOR_NAME[name] = row
    dve_ops.CUSTOM_DVE_SPECS[name] = spec
    _EXP_OP = op
    return op


def _bcast_rows(ap: bass.AP, nrows: int) -> bass.AP:
    """[1, n] AP -> partition-stride-0 [nrows, n] AP (for DMA replicate)."""
    return bass.AP(tensor=ap.tensor, offset=ap.offset, ap=[[0, nrows], ap.ap[-1]])


def build_program():
    exp_op = _register_exp_op()
    sq_op = _register_sq_op()
    nc = bacc.Bacc("TRN2", target_bir_lowering=False, debug=False)

    # rows 0-31: x head-slice; rows 32-63: ones (fills qa/ka ones-rows)
    xh_d = nc.dram_tensor("xh", [B, 2 * C, N], BF16, kind="ExternalInput").ap()
    xt_d = nc.dram_tensor("xt", [B, 128, NT, 33], BF16, kind="ExternalInput").ap()
    wall_d = nc.dram_tensor("wall", [C, 96], BF16, kind="ExternalInput").ap()
    w2_d = nc.dram_tensor("w2t", [C, C], F32R, kind="ExternalInput").ap()
    gm_d = nc.dram_tensor("gm", [C, 1], F32, kind="ExternalInput").ap()
    bt_d = nc.dram_tensor("bt", [C, 1], F32, kind="ExternalInput").ap()
    y_d = nc.dram_tensor("y", [B, C, N], F32, kind="ExternalOutput").ap()

    with tile.TileContext(nc) as tc:
        with tc.tile_pool(name="const", bufs=1) as const, \
             tc.tile_pool(name="persist", bufs=1) as persist, \
             tc.tile_pool(name="drp", bufs=2, space="DRAM") as drp:
            wall_s = const.tile([C, 96], BF16)
            w2_s = const.tile([C, C], F32R)
            gm_s = const.tile([C, 1], F32)
            bt_s = const.tile([C, 1], F32)
            actb = const.tile([128, 1], F32)
            epsb = const.tile([C, 1], F32)
            ones1_s = const.tile([1, C], F32)
            nc.vector.memset(ones1_s, 1.0)
            for dst, src in ((wall_s, wall_d), (w2_s, w2_d),
                             (gm_s, gm_d), (bt_s, bt_d)):
                nc.sync.dma_start(out=dst, in_=src)
            nc.vector.memset(actb, ACT_BIAS)
            nc.vector.memset(epsb, EPS_BN)

            outn_sb = [persist.tile([C, N], F32R, tag=f"on{b}", name=f"outn{b}")
                       for b in range(B)]
            stats = persist.tile([C, B * NC, 6], F32)

            # Emit both batches' prologues first; the Tile scheduler overlaps
            # b=1 prologue with b=0 main loop.
            pro = {}
            es = ExitStack()
            xbp = es.enter_context(tc.tile_pool(name="xb", bufs=1))
            pprojp = es.enter_context(tc.tile_pool(name="pproj", bufs=2,
                                                   space="PSUM"))
            for b in range(B):
                qa = xbp.tile([96, N], BF16, tag=f"qa{b}", name=f"qa{b}")
                ka = xbp.tile([96, N], BF16, tag=f"ka{b}", name=f"ka{b}")
                xaT = xbp.tile([128, NT, 33], BF16, tag=f"xaT{b}",
                               name=f"xaT{b}")
                # x straight into qa's qk-rows; ones rows from the same DMA
                # source; x^T (with ones col) from host
                nc.sync.dma_start(out=qa[0:32, :], in_=xh_d[b][0:32, :])
                nc.sync.dma_start(out=qa[64:96, :], in_=xh_d[b][32:64, :])
                nc.sync.dma_start(out=ka[32:64, :], in_=xh_d[b][32:64, :])
                nc.sync.dma_start(out=xaT, in_=xt_d[b])
                # combined projection: rows 0-31 g = W^T x, 32-63 q, 64-95 k.
                # All PSUM->SBUF prologue work goes to the DVE: it idles
                # during batch-0 prologue and has slack vs ACT in the main
                # loop (ACT is the bottleneck engine).
                for icx in range(NC):
                    sl = bass.ts(icx, 512)
                    ps_all = pprojp.tile([96, 512], F32, tag="pp",
                                         name="ps_all")
                    nc.tensor.matmul(ps_all, lhsT=wall_s, rhs=qa[0:32, sl],
                                     start=True, stop=True)
                    # g-copy on DVE (base-0 partitions only: custom DVE ops
                    # and stock ops misbehave at nonzero base partition);
                    # squares stay on ACT which handles partition offsets.
                    nc.vector.tensor_copy(ka[0:32, sl], ps_all[0:32, :])
                    nc.scalar.activation(qa[32:64, sl], ps_all[32:64, :],
                                         AFT.Square, bias=0.0, scale=1.0)
                    nc.scalar.activation(ka[64:96, sl], ps_all[64:96, :],
                                         AFT.Square, bias=0.0, scale=1.0)
                pro[b] = (qa, ka, xaT)

            # main attention loops: i-blocks of 1024 (2 psum banks)
            with tc.tile_pool(name="mt", bufs=3) as mt, \
                 tc.tile_pool(name="ps_st", bufs=2, space="PSUM") as ps_st, \
                 tc.tile_pool(name="ps_acc", bufs=1, space="PSUM") as ps_acc, \
                 tc.tile_pool(name="ep", bufs=2) as ep:
                for b in range(B):
                    qa, ka, xaT = pro[b]
                    for icb in range(NB):
                        outX_ps = ps_acc.tile([33, 1024], F32, tag="outX",
                                              name="outX_ps")

                        # software-pipelined: PV matmuls for pair k are
                        # emitted after pair k+1's ST matmuls so the PE queue
                        # never stalls waiting on the ACT->DVE chain.
                        pending = []

                        def flush_pv(outX_ps=outX_ps):
                            pj, p_sb = pending.pop(0)
                            for half in range(2):
                                jt = pj * 2 + half
                                for hh in range(2):
                                    nc.tensor.matmul(
                                        outX_ps[:, bass.ts(hh, 512)],
                                        lhsT=xaT[:, jt, :],
                                        rhs=p_sb[:, bass.ds(half * 1024 +
                                                            hh * 512, 512)],
                                        start=(jt == 0),
                                        stop=(jt == NT - 1))

                        for pj in range(NT // 2):
                            t_sb = mt.tile([128, 2048], F32, tag="t",
                                           name="t_sb", bufs=2)
                            for half in range(2):
                                jt = pj * 2 + half
                                st_ps = ps_st.tile([128, 1024], F32, tag="st",
                                                   name="st_ps")
                                for hh in range(2):
                                    nsl = bass.ds(icb * 1024 + hh * 512, 512)
                                    nc.tensor.matmul(
                                        st_ps[:, bass.ts(hh, 512)],
                                        lhsT=ka[:, bass.ts(jt, 128)],
                                        rhs=qa[:, nsl],
                                        start=True, stop=True)
                                nc.scalar.activation(
                                    t_sb[:, bass.ts(half, 1024)], st_ps,
                                    AFT.Sqrt, bias=actb, scale=ACT_SCALE)
                            p_sb = mt.tile([128, 2048], BF16, tag="p",
                                           name="p_sb", bufs=2)
                            nc.vector._custom_dve(exp_op, out=p_sb, in0=t_sb,
                                                  s0=B0, s1=B1, imm2=B2)
                            pending.append((pj, p_sb))
                            if len(pending) > 1:
                                flush_pv()
                        while pending:
                            flush_pv()
                        # epilogue for this i-block: one ACT pass brings outX
                        # (incl rowsum row) to SBUF; recip runs on the [128,8]
                        # layout via DMA reshape; normalize on GPSIMD.
                        outc = ep.tile([33, 1024], F32, tag="outc",
                                       name="outc")
                        nc.scalar.copy(outc, outX_ps)
                        r128 = ep.tile([128, 8], F32, tag="r128", name="r128")
                        nc.sync.dma_start(out=r128, in_=outc[32:33, :])
                        recip = ep.tile([128, 8], F32, tag="recip",
                                        name="recip")
                        nc.vector.reciprocal(recip, r128)
                        last = (b == B - 1 and icb == NB - 1)
                        if not last:
                            rdr = drp.tile([1, 1024], F32, tag="rdr",
                                           name="rdr")
                            nc.sync.dma_start(out=rdr, in_=recip)
                            rbc = ep.tile([C, 1024], F32, tag="rbc",
                                          name="rbc")
                            nc.sync.dma_start(out=rbc, in_=_bcast_rows(rdr, C))
                            osl = bass.ds(icb * 1024, 1024)
                            nc.gpsimd.tensor_mul(outn_sb[b][:, osl],
                                                 outc[0:32, :], rbc)
                        else:
                            # final block: shortest-latency path (PE bcast,
                            # DVE mul) since nothing overlaps the tail
                            rr1 = ep.tile([1, 1024], F32, tag="rr1",
                                          name="rr1")
                            nc.sync.dma_start(out=rr1, in_=recip)
                        for hh in range(2):
                            hsl = bass.ds(icb * 1024 + hh * 512, 512)
                            if last:
                                rb_ps = pprojp.tile([C, 512], F32, tag="pp",
                                                    name="rb_ps")
                                nc.tensor.matmul(rb_ps, lhsT=ones1_s,
                                                 rhs=rr1[:, bass.ts(hh, 512)],
                                                 start=True, stop=True)
                                nc.vector.tensor_mul(
                                    outn_sb[b][:, hsl],
                                    outc[0:32, bass.ts(hh, 512)], rb_ps)
                            y_ps = pprojp.tile([C, 512], F32, tag="pp",
                                               name="y_ps")
                            nc.tensor.matmul(y_ps, lhsT=w2_s,
                                             rhs=outn_sb[b][:, hsl],
                                             start=True, stop=True)
                            nc.vector.bn_stats(
                                stats[:, (b * NB + icb) * 2 + hh, :], y_ps)
            es.close()

            # BatchNorm tail: aggregate stats, re-project, affine, store
            with tc.tile_pool(name="tail", bufs=1) as tail, \
                 tc.tile_pool(name="ps_tl", bufs=2, space="PSUM") as ps_tl:
                mv = tail.tile([C, 2], F32)
                nc.vector.bn_aggr(mv, stats)
                std = tail.tile([C, 1], F32)
                nc.scalar.activation(std, mv[:, 1:2], AFT.Sqrt,
                                     bias=epsb, scale=1.0)
                rstd = tail.tile([C, 1], F32)
                nc.vector.reciprocal(rstd, std)
                sc = tail.tile([C, 1], F32)
                nc.vector.tensor_mul(sc, gm_s, rstd)
                msc = tail.tile([C, 1], F32)
                nc.vector.tensor_mul(msc, mv[:, 0:1], sc)
                nb = tail.tile([C, 1], F32)
                nc.vector.tensor_sub(nb, bt_s, msc)
                for b in range(B):
                    for icx in range(NC):
                        sl = bass.ts(icx, 512)
                        yt_ps = ps_tl.tile([C, 512], F32, tag="yt",
                                           name="yt_ps")
                        nc.tensor.matmul(yt_ps, lhsT=w2_s,
                                         rhs=outn_sb[b][:, sl],
                                         start=True, stop=True)
                        yo = tail.tile([C, 512], F32, tag="yo", name="yo",
                                       bufs=4)
                        if icx % 2 == 0:
                            nc.scalar.activation(yo, yt_ps, AFT.Identity,
                                                 bias=nb, scale=sc)
                        else:
                            nc.vector.tensor_scalar(
                                out=yo, in0=yt_ps, scalar1=sc, scalar2=nb,
                                op0=mybir.AluOpType.mult,
                                op1=mybir.AluOpType.add)
                        nc.sync.dma_start(out=y_d[b][:, sl], in_=yo)

    nc.compile()
    return nc


_NC_CACHE = None


def _get_nc():
    global _NC_CACHE
    if _NC_CACHE is None:
        _NC_CACHE = build_program()
    return _NC_CACHE


def make_in_maps(x, wq, wk, wv, wo, gamma, beta):
    import ml_dtypes
    f = np.float32
    bf = ml_dtypes.bfloat16
    in_maps = []
    for h in range(H):
        cs = slice(h * C, (h + 1) * C)
        xh = np.ones((B, 2 * C, N), f)
        xh[:, 0:C, :] = x[:, cs, :]
        xh = xh.astype(bf)
        # x^T tiles with ones column: [B, 128, NT, 33]
        xt = np.ones((B, 128, NT, 33), f)
        xtr = np.ascontiguousarray(x[:, cs, :].transpose(0, 2, 1))  # [B,N,C]
        xt[:, :, :, :32] = xtr.reshape(B, NT, 128, C).transpose(0, 2, 1, 3)
        wall = np.empty((C, 96), f)
        wall[:, 0:32] = -2.0 * (wk[h].T @ wq[h])    # g-rows weight (G.T)
        wall[:, 32:64] = wq[h].T
        wall[:, 64:96] = wk[h].T
        in_maps.append({
            "xh": xh,
            "xt": xt.astype(bf),
            "wall": wall.astype(bf),
            "w2t": np.ascontiguousarray((wo[h] @ wv[h]).T.astype(f)),
            "gm": np.ascontiguousarray(gamma[cs].reshape(C, 1).astype(f)),
            "bt": np.ascontiguousarray(beta[cs].reshape(C, 1).astype(f)),
        })
    return in_maps


def kernel(x, wq, wk, wv, wo, bo, gamma, beta):
    x, wq, wk, wv, wo, gamma, beta = (np.asarray(a) for a in
                                      (x, wq, wk, wv, wo, gamma, beta))
    nc = _get_nc()
    in_maps = make_in_maps(x, wq, wk, wv, wo, gamma, beta)
    res = run_bass_kernel_spmd(nc, in_maps, list(range(H)))
    y = np.empty((B, DIM, N), np.float32)
    for h in range(H):
        y[:, h * C:(h + 1) * C, :] = res.results[h]["y"]
    return y
